# revision 8
# baseline (speedup 1.0000x reference)
"""Trainium2 Bass kernel for CustomMultiHeadAttention (B=4, S=1024, D=1024, H=16, Dh=64).

Sharding: 8 cores = (batch b in 0..3) x (head-group g in 0..1).
Core (b, g) computes heads 8g..8g+7 of batch b over the FULL sequence:
Q/K/V projections use only the group's 512 columns of Wq/Wk/Wv, the
output projection contracts the group's 512 rows of Wo, producing a
partial [S, D] output; the host sums the two partials per batch (+bo).
Nothing is computed twice across cores, and per-core input DMA drops
to ~6.7 MB (the old (b, parity) scheme moved 12.9 MB and duplicated
the full-sequence K/V projections in each parity pair).

Pipeline (transposed-layout, PE-centric, same tricks as the parity
kernel): QT/KT rope via permutation-matmul + DVE, scores s[kv,q] per
head with psum chunks <=512, exp on ScalarE (scale=1/8), causal
triangle mask as a f16 0/1 multiply on the 128 diagonal cols,
ctx/denoms via lhsT=[V|1] slots, normalize via reciprocal_approx_fast
+ gpsimd broadcast, partial out = cn^T Wo_half (no bias; host adds bo).
"""

import threading

import numpy as np

B, S, D, H, Dh = 4, 1024, 1024, 16, 64
P = 128
N_CORES = 8
NT = D // P        # 8 tiles along the model dim
HG = 8             # heads per core
QT_T = 4           # qt/kt dout tiles per core (2 heads each)
VS = 65            # V slot width: [V(64) | ones(1)] per head

_cache = {}
_lock = threading.Lock()


def _build_program(taps=False):
    import concourse.bass as bass  # noqa: F401
    import concourse.mybir as mybir
    import concourse.tile as tile
    from concourse import bacc

    dt = mybir.dt
    f16, f32 = dt.float16, dt.float32
    AF = mybir.ActivationFunctionType

    nc = bacc.Bacc("TRN2", target_bir_lowering=False, debug=False,
                   num_devices=N_CORES)

    def ein(name, shape):
        return nc.dram_tensor(name, shape, f16, kind="ExternalInput").ap()

    xt_e = ein("xt", [P, NT, S])          # x[b]^T, host-transposed
    wq_e = ein("wq", [P, NT, 512])        # Wq[:, half], host-tiled
    wk_e = ein("wk", [P, NT, 512])
    wv_e = ein("wv", [P, NT, 512])
    wo_e = ein("wo", [P, QT_T, D])        # Wo[half, :], host-tiled
    bqt_e = nc.dram_tensor("bqt", [P, QT_T], f32, kind="ExternalInput").ap()
    bkt_e = nc.dram_tensor("bkt", [P, QT_T], f32, kind="ExternalInput").ap()
    bvb_e = ein("bvb", [P, 512])          # bv[half] broadcast across parts
    cos_e = ein("cosk", [P, S])
    sin_e = ein("sink", [P, S])
    m128_e = ein("m128", [P, P])          # causal triangle (r<=c)
    p128_e = ein("p128", [P, P])          # rope xor-32 permutation
    y_e = nc.dram_tensor("y_sh", [S, D], f16, kind="ExternalOutput").ap()
    tap_ext = {}
    if taps:
        for tn, shape in (("qt", [P, QT_T, S]), ("kt", [P, QT_T, S]),
                          ("v1", [P, NT, HG * VS]), ("cn", [P, QT_T, S])):
            tap_ext[tn] = nc.dram_tensor("dbg_" + tn, shape, f16,
                                         kind="ExternalOutput").ap()

    with tile.TileContext(nc) as tc:
        from contextlib import ExitStack
        with ExitStack() as ctx:
            big = ctx.enter_context(tc.tile_pool(name="big", bufs=1))

            xT = big.tile([P, NT, S], f16, tag="xT")
            wq = big.tile([P, NT, 512], f16, tag="wq")
            wk = big.tile([P, NT, 512], f16, tag="wk")
            wv = big.tile([P, NT, 512], f16, tag="wv")
            wo = big.tile([P, QT_T, D], f16, tag="wo")
            bqt = big.tile([P, QT_T], f32, tag="bqt")
            bkt = big.tile([P, QT_T], f32, tag="bkt")
            bvb = big.tile([P, 512], f16, tag="bvb")
            qt = big.tile([P, QT_T, S], f16, tag="qt")    # rope'd Q^T
            kt = big.tile([P, QT_T, S], f16, tag="kt")    # rope'd K^T
            v1 = big.tile([P, NT, HG * VS], f16, tag="v1")
            cn = big.tile([P, QT_T, S], f16, tag="cn")    # normalized ctx^T
            cos = big.tile([P, S], f16, tag="cos")
            sin = big.tile([P, S], f16, tag="sin")
            m128 = big.tile([P, P], f16, tag="m128")
            p128 = big.tile([P, P], f16, tag="p128")

            # ---- input DMAs ----
            # Four queues pull in parallel; per-queue order matches first
            # use. Every tensor is host-packed to >=2KB contiguous lines.
            for t, e in ((cos, cos_e), (sin, sin_e), (bqt, bqt_e),
                         (bkt, bkt_e), (p128, p128_e)):
                nc.scalar.dma_start(t[:], e[:])
            nc.gpsimd.dma_start(wq[:, 0:4, :], wq_e[:, 0:4, :])
            nc.sync.dma_start(xT[:, 0:4, :], xt_e[:, 0:4, :])
            nc.gpsimd.dma_start(wq[:, 4:8, :], wq_e[:, 4:8, :])
            nc.sync.dma_start(xT[:, 4:8, :], xt_e[:, 4:8, :])
            nc.scalar.dma_start(wk[:], wk_e[:])
            nc.scalar.dma_start(m128[:], m128_e[:])
            nc.gpsimd.dma_start(wv[:], wv_e[:])
            nc.gpsimd.dma_start(bvb[:], bvb_e[:])
            nc.sync.dma_start(wo[:], wo_e[:])
            # ones columns of the V slots (col 64 of each 65-wide slot)
            v1r = v1.rearrange("p t (h c) -> p t h c", c=VS)
            for t in range(NT):
                nc.any.memset(v1r[:, t, :, 64:65], 1.0)

            with tc.tile_pool(name="pp", bufs=2, space="PSUM") as pp, \
                 tc.tile_pool(name="sc", bufs=4) as sc, \
                 tc.tile_pool(name="scp", bufs=2, space="PSUM") as scp, \
                 tc.tile_pool(name="cxp", bufs=1, space="PSUM") as cxp, \
                 tc.tile_pool(name="ep", bufs=3) as ep, \
                 tc.tile_pool(name="npl", bufs=2) as npl, \
                 tc.tile_pool(name="ysp", bufs=2) as ysp:
                pending = []

                def flush_one():
                    dst, raw, cos_ap, sin_ap = pending.pop(0)
                    pq = pp.tile([P, 512], f32, tag="ps", name="pq")
                    nc.tensor.matmul(pq[:], p128[:], raw[:],
                                     start=True, stop=True)
                    t1 = sc.tile([P, 512], f16, tag="t1", name="t1")
                    nc.vector.tensor_mul(t1[:], raw[:], cos_ap)
                    t2 = sc.tile([P, 512], f16, tag="t2", name="t2")
                    nc.vector.tensor_mul(t2[:], pq[:], sin_ap)
                    nc.vector.tensor_add(dst, t1[:], t2[:])

                def rope_chain(dst, w_sb, bias_col, t, csl):
                    ps = pp.tile([P, 512], f32, tag="ps", name="ps")
                    for k in range(NT):
                        nc.tensor.matmul(ps[:], w_sb[:, k, P * t:P * (t + 1)],
                                         xT[:, k, csl], start=(k == 0),
                                         stop=(k == NT - 1))
                    # psum->sbuf f16 with fused per-partition bias (DVE,
                    # keeping ScalarE free for the attention exps)
                    raw = sc.tile([P, 512], f16, tag="raw", name="raw")
                    nc.vector.tensor_scalar_add(raw[:], ps[:], bias_col)
                    pending.append((dst, raw, cos[:, csl], sin[:, csl]))
                    if len(pending) > 1:
                        flush_one()

                def emit_q(t):
                    for n in range(2):
                        csl = slice(512 * n, 512 * (n + 1))
                        rope_chain(qt[:, t, csl], wq, bqt[:, t:t + 1], t, csl)

                def emit_k(t):
                    for n in range(2):
                        csl = slice(512 * n, 512 * (n + 1))
                        rope_chain(kt[:, t, csl], wk, bkt[:, t:t + 1], t, csl)

                def emit_v(t):
                    # V s-tile t: natural [s, dout_half] into 65-wide slots
                    ssl = slice(P * t, P * (t + 1))
                    vp = pp.tile([P, 512], f32, tag="ps", name="vp")
                    for k in range(NT):
                        nc.tensor.matmul(vp[:], xT[:, k, ssl], wv[:, k, :],
                                         start=(k == 0), stop=(k == NT - 1))
                    nc.vector.tensor_add(
                        v1r[:, t, :, 0:64],
                        vp.rearrange("p (h c) -> p h c", c=64),
                        bvb.rearrange("p (h c) -> p h c", c=64))

                def emit_attn(h, fillers=False):
                    # head h: scores s[kv, q] per kv-block j in a 2-bank
                    # psum tile (bank0: first 512 active q cols, bank1: the
                    # rest), ONE exp per (h, j), causal triangle on the 128
                    # diagonal cols, ctx+denom via the [V|1] slot.
                    th, hp = h // 2, h % 2
                    rsl = slice(64 * hp, 64 * hp + 64)
                    cxL = cxp.tile([VS, 512], f32, tag="cxL", name="cxL")
                    cxR = cxp.tile([VS, 512], f32, tag="cxR", name="cxR")
                    es = {}

                    def filler():
                        # tiny independent matmul: keeps the PE active (HAM
                        # at K=8) while ctx waits on ScalarE exps
                        fp = pp.tile([P, 512], f32, tag="ps", name="fil")
                        nc.tensor.matmul(fp[:, 0:256], warm[:, 0:P],
                                         warm[:, 0:256], start=True,
                                         stop=True)

                    def emit_scores(j):
                        e = ep.tile([P, S], f16, tag="e", name=f"e{h}_{j}")
                        N = S - P * j
                        ksl = slice(P * j, P * (j + 1))
                        sw = scp.tile([P, S], f32, tag="s",
                                      name=f"s{h}_{j}")
                        if j < 4:
                            # bank0: q cols [128j, 128j+512)
                            nc.tensor.matmul(sw[:, 0:512], kt[rsl, th, ksl],
                                             qt[rsl, th, P * j:P * j + 512],
                                             start=True, stop=True,
                                             skip_group_check=True)
                            # bank1: q cols [128j+512, 1024)
                            nc.tensor.matmul(sw[:, 512:N], kt[rsl, th, ksl],
                                             qt[rsl, th, P * j + 512:1024],
                                             start=True, stop=True,
                                             skip_group_check=True)
                        else:
                            nc.tensor.matmul(sw[:, 0:N], kt[rsl, th, ksl],
                                             qt[rsl, th, P * j:1024],
                                             start=True, stop=True,
                                             skip_group_check=True)
                        nc.scalar.activation(e[:, 0:N], sw[:, 0:N],
                                             AF.Exp, scale=0.125)
                        # causal triangle on the diagonal block
                        eng = nc.vector if j % 2 == 0 else nc.gpsimd
                        eng.tensor_mul(e[:, 0:P], e[:, 0:P], m128[:])
                        es[j] = e

                    def emit_ctx(j):
                        e = es.pop(j)
                        slot = v1[:, j, VS * h:VS * h + VS]
                        if j < 4:
                            wa = 512 - P * j
                            nc.tensor.matmul(cxL[:, P * j:512], slot,
                                             e[:, 0:wa], start=(j == 0),
                                             stop=(j == 3))
                            nc.tensor.matmul(cxR[:], slot, e[:, wa:wa + 512],
                                             start=(j == 0), stop=(j == 7))
                        else:
                            N = S - P * j
                            nc.tensor.matmul(cxR[:, P * j - 512:512], slot,
                                             e[:, 0:N], start=False,
                                             stop=(j == 7))

                    # depth-2 software pipeline: scores run ahead of ctx
                    for j in range(NT + 2):
                        if j < NT:
                            emit_scores(j)
                        if j >= 2:
                            if fillers:
                                filler()
                            emit_ctx(j - 2)

                    # normalize: denom row -> recip -> gpsimd broadcast
                    dd = npl.tile([1, S], f32, tag="d", name="dd")
                    nc.vector.tensor_copy(dd[:, 0:512], cxL[64:65, :])
                    nc.vector.tensor_copy(dd[:, 512:1024], cxR[64:65, :])
                    rr = npl.tile([1, S], f32, tag="r", name="rr")
                    nc.vector.reciprocal_approx_fast(rr[:], dd[:])
                    rbs = npl.tile([64, S], f32, tag="rb", name="rbs")
                    nc.gpsimd.partition_broadcast(rbs[:], rr[:], channels=64)
                    nc.vector.tensor_mul(cn[rsl, th, 0:512], cxL[0:64, :],
                                         rbs[:, 0:512])
                    nc.vector.tensor_mul(cn[rsl, th, 512:1024], cxR[0:64, :],
                                         rbs[:, 512:1024])

                # Dummy matmuls at the head of the PE queue: keep the PE
                # array busy while the first DMAs land so the HAM clock
                # gate opens before the real chains start.
                warm = sc.tile([P, 512], f16, tag="warm", name="warm")
                nc.vector.memset(warm[:], 0.0)
                for i in range(28):
                    wp = pp.tile([P, 512], f32, tag="ps", name="wp")
                    nc.tensor.matmul(wp[:], warm[:, 0:P], warm[:],
                                     start=True, stop=True)

                # Emission order tracks DMA arrival: Q chains first (xT +
                # wq + consts), then V tiles (wv), then K staged with
                # attention heads interleaved so ScalarE exp overlaps the
                # remaining projection matmuls.
                for t in range(QT_T):
                    emit_q(t)
                for t in range(4):
                    emit_v(t)
                emit_k(0)
                for t in range(4, NT):
                    emit_v(t)
                emit_k(1)
                emit_attn(0)
                emit_k(2)
                emit_attn(1)
                emit_k(3)
                emit_attn(2)
                while pending:
                    flush_one()
                for h in range(3, HG):
                    emit_attn(h, fillers=True)

                # ---- partial out-projection ----
                for i in range(NT):
                    ys = ysp.tile([P, S], f16, tag="ys", name=f"ys{i}")
                    for n in range(2):
                        yp = pp.tile([P, 512], f32, tag="ps",
                                     name=f"yp{i}_{n}")
                        csl = slice(512 * n, 512 * (n + 1))
                        for t in range(QT_T):
                            nc.tensor.matmul(yp[:],
                                             cn[:, t, P * i:P * (i + 1)],
                                             wo[:, t, csl],
                                             start=(t == 0),
                                             stop=(t == QT_T - 1))
                        if n == 0:
                            nc.vector.tensor_copy(ys[:, csl], yp[:])
                        else:
                            nc.scalar.copy(ys[:, csl], yp[:])
                    q_eng = (nc.sync, nc.scalar, nc.gpsimd)[i % 3]
                    q_eng.dma_start(y_e[P * i:P * (i + 1), :], ys[:])

            if taps:
                for tn, tile_ap in (("qt", qt), ("kt", kt), ("v1", v1),
                                    ("cn", cn)):
                    nc.sync.dma_start(tap_ext[tn][:], tile_ap[:])

    nc.compile()
    return nc


def _host_tables():
    # RoPE tables, computed in float32 to match the reference's jnp path.
    pos = np.arange(S, dtype=np.float32)
    inv = np.exp(np.arange(0, Dh, 2, dtype=np.float32)
                 * np.float32(-np.log(10000.0) / Dh))          # [32]
    ang = pos[:, None] * inv[None, :]                          # [S, 32]
    sin = np.sin(ang).astype(np.float32)
    cos = np.cos(ang).astype(np.float32)
    # per-partition pattern for [2 heads x 64, s] transposed layout
    dd = np.arange(P) % Dh
    cosP = np.empty((P, S), np.float32)
    sinP = np.empty((P, S), np.float32)
    lo = dd < 32
    cosP[lo] = cos[:, dd[lo]].T
    sinP[lo] = -sin[:, dd[lo]].T
    cosP[~lo] = cos[:, dd[~lo] - 32].T
    sinP[~lo] = sin[:, dd[~lo] - 32].T
    return cosP.astype(np.float16), sinP.astype(np.float16)


def _perm128():
    p = np.zeros((P, P), np.float16)
    i = np.arange(P)
    p[i, i ^ 32] = np.float16(1.0)
    return p


def _tile_T(a):
    # [rows, D] -> [P, NT, rows]: partition-tiled transpose for SBUF layout
    rows = a.shape[0]
    return np.ascontiguousarray(a.T.reshape(NT, P, rows).transpose(1, 0, 2))


def _w_half(w, g):
    # Wx[:, 512g:512(g+1)] -> [P, NT, 512] in SBUF layout (contiguous)
    h = np.asarray(w, np.float16)[:, 512 * g:512 * (g + 1)]
    return np.ascontiguousarray(h.reshape(NT, P, 512).transpose(1, 0, 2))


def _wo_half(w, g):
    # Wo[512g:512(g+1), :] -> [P, QT_T, D] in SBUF layout (contiguous)
    h = np.asarray(w, np.float16)[512 * g:512 * (g + 1), :]
    return np.ascontiguousarray(h.reshape(QT_T, P, D).transpose(1, 0, 2))


def _b_half(b, g):
    h = np.asarray(b, np.float16).astype(np.float32)[512 * g:512 * (g + 1)]
    return np.ascontiguousarray(h.reshape(QT_T, P).T)


def make_in_maps(x, Wq, bq, Wk, bk, Wv, bv, Wo, bo):
    x = np.asarray(x, np.float16)
    cosP, sinP = _host_tables()
    r = np.arange(P)[:, None]
    c = np.arange(P)[None, :]
    m128 = (r <= c).astype(np.float16)
    p128 = _perm128()

    halves = []
    for g in range(2):
        halves.append({
            "wq": _w_half(Wq, g), "wk": _w_half(Wk, g),
            "wv": _w_half(Wv, g), "wo": _wo_half(Wo, g),
            "bqt": _b_half(bq, g), "bkt": _b_half(bk, g),
            "bvb": np.ascontiguousarray(np.broadcast_to(
                np.asarray(bv, np.float16)[512 * g:512 * (g + 1)]
                .reshape(1, 512), (P, 512))),
        })

    in_maps = []
    for core in range(N_CORES):
        b, g = core // 2, core % 2
        m = {
            "xt": _tile_T(x[b]),
            "cosk": cosP, "sink": sinP,
            "m128": m128, "p128": p128,
        }
        m.update(halves[g])
        in_maps.append(m)
    return in_maps


def kernel(x, Wq, bq, Wk, bk, Wv, bv, Wo, bo):
    from concourse.bass_utils import run_bass_kernel_spmd

    with _lock:
        if "nc" not in _cache:
            _cache["nc"] = _build_program()
    nc = _cache["nc"]

    in_maps = make_in_maps(x, Wq, bq, Wk, bk, Wv, bv, Wo, bo)
    res = run_bass_kernel_spmd(nc, in_maps, list(range(N_CORES)))

    bo32 = np.asarray(bo, np.float16).astype(np.float32)
    out = np.empty((B, S, D), np.float16)
    for b in range(B):
        acc = res.results[2 * b]["y_sh"].astype(np.float32)
        acc += res.results[2 * b + 1]["y_sh"].astype(np.float32)
        out[b] = (acc + bo32).astype(np.float16)
    return out
